# revision 1
# baseline (speedup 1.0000x reference)
"""CrossConv2d (concat -> 3x3 conv -> BN -> +skip -> ReLU) on 8 Trainium2 cores.

Data-parallel over the fused (b*s)=32 batch axis: 4 images per core.
Per-core Bass/Tile kernel:
  - channels (64 u + 64 v = 128) live on SBUF partitions
  - inputs are zero-padded host-side to (H+2) x (W+2), flattened, and
    pre-rounded to the fp32r grid (round-half-up to 11 mantissa bits),
    so each strip stage is one fully contiguous fp32r DMA per channel
    half straight into the matmul operand tile; the 3x3 conv is 9
    shifted matmuls (lhsT = W[tap] as [C_in, C_out], fp32r)
    accumulating into PSUM over 462-pixel chunks
  - whole padded images live in two persistent SBUF tiles (ping-pong);
    u is loaded only twice per core (all 4 images share it), v refilled
    per image; output is written width-padded (W+2) and sliced
    host-side, keeping the store DMA contiguous too
  - BN scale folded into the conv weights host-side; BN shift applied as
    the ScalarE Relu bias; skip-add is one VectorE add (in-place in PSUM)
"""

import numpy as np

import concourse.bacc as bacc
import concourse.mybir as mybir
from concourse import tile
from concourse.bass_utils import run_bass_kernel_spmd

EPS = 1e-5

B, S, C1, C2, H, W = 4, 8, 64, 64, 128, 128
CC = C1 + C2  # 128 concat channels = out channels = partition count
N_CORES = 8
IMG_PER_CORE = (B * S) // N_CORES  # 4
STRIP = 32                 # output rows per strip
NSTRIPS = H // STRIP
WP = W + 2                 # padded width
HP = H + 2                 # padded height (inputs only)
HALO = STRIP + 2           # input rows staged per strip
CHUNK = 512                # one PSUM bank; image = 31x512 + 2x383 chunks (all >=256)

F32 = mybir.dt.float32
MM_DT = mybir.dt.float32r  # full-rate fp32 matmul mode

_CACHE = {}


def _build_program():
    nc = bacc.Bacc(
        "TRN2", target_bir_lowering=False, debug=False, num_devices=N_CORES
    )
    u_d = nc.dram_tensor("u", [C1, HP * WP], MM_DT, kind="ExternalInput")
    v_d = nc.dram_tensor("v", [IMG_PER_CORE, C2, HP * WP], MM_DT, kind="ExternalInput")
    w_d = nc.dram_tensor("w", [CC, 9 * CC], MM_DT, kind="ExternalInput")
    sh_d = nc.dram_tensor("shift", [CC, 1], F32, kind="ExternalInput")
    o_d = nc.dram_tensor("o", [IMG_PER_CORE, CC, H * WP], F32, kind="ExternalOutput")

    with tile.TileContext(nc) as tc:
        with (
            tc.tile_pool(name="consts", bufs=1) as cpool,
            tc.tile_pool(name="ostrip", bufs=6) as opool,
            tc.tile_pool(name="psum", bufs=8, space="PSUM") as ppool,
        ):
            # two persistent whole-image tiles (ping-pong): all 4
            # images share the same u (same b), so the u half is loaded
            # only once per tile; only the v half is re-DMA'd per image.
            # One leading pad element aligns every PSUM chunk to an even
    	    # 512 boundary (out flat q reads input at 1 + q + off).
            xr_a = cpool.tile([CC, HP * WP + 4], MM_DT)
            xr_b = cpool.tile([CC, HP * WP + 4], MM_DT)
            # startup critical path: only xr_a's leading pad cells and
            # tap-0 weights gate the first matmul group — issue those
            # first, tap weights in use order, the rest afterwards
            nc.scalar.dma_start(xr_a[0:C1, 0:4], u_d[:, 0:4])
            nc.scalar.dma_start(xr_a[C1:CC, 0:4], v_d[0, :, 0:4])
            w_r = cpool.tile([CC, 9 * CC], MM_DT)
            for t in range(9):
                nc.scalar.dma_start(
                    w_r[:, t * CC : (t + 1) * CC], w_d[:, t * CC : (t + 1) * CC]
                )
            sh_sb = cpool.tile([CC, 1], F32)
            nc.scalar.dma_start(sh_sb[:], sh_d[:])
            nc.scalar.dma_start(xr_b[0:C1, 0:4], u_d[:, 0:4])
            nc.scalar.dma_start(xr_b[C1:CC, 0:4], v_d[0, :, 0:4])
            for t_ in (xr_a, xr_b):
                nc.scalar.dma_start(t_[0:C1, 1 + HP * WP :], u_d[:, 0:3])
                nc.scalar.dma_start(t_[C1:CC, 1 + HP * WP :], v_d[0, :, 0:3])
            NBLK = 16
            blk = [(HP * WP * k // NBLK, HP * WP * (k + 1) // NBLK)
                   for k in range(NBLK)]

            for img in range(IMG_PER_CORE):
                xr = xr_a if img % 2 == 0 else xr_b
                for b0, b1 in blk:
                    if img < 2:
                        nc.sync.dma_start(
                            xr[0:C1, 1 + b0 : 1 + b1], u_d[:, b0:b1]
                        )
                    nc.sync.dma_start(
                        xr[C1:CC, 1 + b0 : 1 + b1], v_d[img, :, b0:b1]
                    )

                # 32 x 512 + 256: covers [0, 16640) incl. junk pad columns
                NQ = H * WP
                starts = [CHUNK * k for k in range(32)] + [32 * CHUNK]
                chunks = [(st, min(st + CHUNK, NQ)) for st in starts]
                # taps-outer over groups of 3 chunks; output DMA per group
                pss = {}
                for g0 in range(0, len(chunks), 3):
                    grp = chunks[g0 : g0 + 3]
                    gq0, gq1 = grp[0][0], grp[-1][1]
                    ogrp = opool.tile([CC, 3 * CHUNK], F32, tag="og")
                    for c, _ in grp:
                        ps_g = ppool.tile([CC, CHUNK], F32, tag="ps")
                        pss[c] = ps_g
                    for t in range(9):
                        dy, dx = t // 3 - 1, t % 3 - 1
                        off = 1 + (1 + dy) * WP + dx
                        for qc0, qc1 in grp:
                            nc.tensor.matmul(
                                pss[qc0][:, 0 : qc1 - qc0],
                                w_r[:, t * CC : (t + 1) * CC],
                                xr[:, qc0 + off : qc1 + off],
                                start=(t == 0),
                                stop=(t == 8),
                            )
                    for qc0, qc1 in grp:
                        n = qc1 - qc0
                        ps = pss[qc0]
                        # skip-add: out flat q reads input flat q + WP
                        nc.vector.tensor_add(
                            ps[:, 0:n], ps[:, 0:n],
                            xr[:, 1 + qc0 + WP : 1 + qc1 + WP],
                        )
                        nc.scalar.activation(
                            ogrp[:, qc0 - gq0 : qc1 - gq0],
                            ps[:, 0:n],
                            mybir.ActivationFunctionType.Relu,
                            bias=sh_sb[:],
                            scale=1.0,
                        )
                    # last image: sync queue (input prefetch is done and
                    # the SWDGE tail drain is ~6us slower to quiesce);
                    # final group goes out per-chunk to shorten the tail
                    last_img = img == IMG_PER_CORE - 1
                    oeng = nc.sync if last_img else nc.gpsimd
                    if last_img and g0 + 3 >= len(chunks):
                        for qc0, qc1 in grp:
                            oeng.dma_start(
                                o_d[img, :, qc0:qc1],
                                ogrp[:, qc0 - gq0 : qc1 - gq0],
                            )
                    else:
                        oeng.dma_start(
                            o_d[img, :, gq0:gq1],
                            ogrp[:, 0 : gq1 - gq0],
                        )
    nc.compile()
    return nc


def _get_program():
    if "nc" not in _CACHE:
        _CACHE["nc"] = _build_program()
    return _CACHE["nc"]


def _round_fp32r(a):
    """Round fp32 array to the fp32r grid: half-up at 11 mantissa bits."""
    bits = np.ascontiguousarray(a, dtype=np.float32).view(np.uint32)
    r = ((bits.astype(np.uint64) + 0x800) & ~np.uint64(0xFFF)).astype(np.uint32)
    return r.view(np.float32)


def _prep_inputs(u, v, conv_w, bn_gamma, bn_beta, bn_mean, bn_var):
    u = np.asarray(u, dtype=np.float32)
    v = np.asarray(v, dtype=np.float32)
    conv_w = np.asarray(conv_w, dtype=np.float32)
    bn_gamma = np.asarray(bn_gamma, dtype=np.float32)
    bn_beta = np.asarray(bn_beta, dtype=np.float32)
    bn_mean = np.asarray(bn_mean, dtype=np.float32)
    bn_var = np.asarray(bn_var, dtype=np.float32)

    scale = bn_gamma / np.sqrt(bn_var + EPS)
    shift = (bn_beta - bn_mean * scale).astype(np.float32).reshape(CC, 1)
    wsc = (conv_w * scale[:, None, None, None]).astype(np.float32)
    # lhsT layout per tap t = ky*3+kx: w_host[i, t*CC + o] = wsc[o, i, ky, kx]
    w_host = _round_fp32r(
        np.ascontiguousarray(wsc.transpose(1, 2, 3, 0).reshape(CC, 9 * CC))
    )

    in_maps = []
    for m in range(N_CORES):
        b = m // 2
        s0 = (m % 2) * IMG_PER_CORE
        u_pad = np.zeros((C1, HP, WP), np.float32)
        u_pad[:, 1 : 1 + H, 1 : 1 + W] = u[b, 0]
        v_pad = np.zeros((IMG_PER_CORE, C2, HP, WP), np.float32)
        v_pad[:, :, 1 : 1 + H, 1 : 1 + W] = v[b, s0 : s0 + IMG_PER_CORE]
        in_maps.append(
            {
                "u": _round_fp32r(u_pad.reshape(C1, HP * WP)),
                "v": _round_fp32r(v_pad.reshape(IMG_PER_CORE, C2, HP * WP)),
                "w": w_host,
                "shift": shift,
            }
        )
    return in_maps


def _run(inputs, trace=False):
    nc = _get_program()
    in_maps = _prep_inputs(**inputs)
    res = run_bass_kernel_spmd(
        nc, in_maps, list(range(N_CORES)), trace=trace
    )
    out = np.empty((B, 1, S, CC, H, W), np.float32)
    for m in range(N_CORES):
        b = m // 2
        s0 = (m % 2) * IMG_PER_CORE
        o_pad = res.results[m]["o"].reshape(IMG_PER_CORE, CC, H, WP)
        out[b, 0, s0 : s0 + IMG_PER_CORE] = o_pad[:, :, :, 1 : 1 + W]
    return out, res


def kernel(u, v, conv_w, bn_gamma, bn_beta, bn_mean, bn_var):
    out, _ = _run(
        dict(
            u=u,
            v=v,
            conv_w=conv_w,
            bn_gamma=bn_gamma,
            bn_beta=bn_beta,
            bn_mean=bn_mean,
            bn_var=bn_var,
        )
    )
    return out



# revision 9
# speedup vs baseline: 1.1613x; 1.1613x over previous
"""CrossConv2d (concat -> 3x3 conv -> BN -> +skip -> ReLU) on 8 Trainium2 cores.

Data-parallel over the fused (b*s)=32 batch axis: 4 images per core.

1D Winograd F(2,3) along x, direct 3-tap accumulation along y, bf16:
  - host side: inputs are padded, cast to bf16, and x-transformed into
    4 Winograd components D[k] (one subtract/add per element); the
    BN-scale-folded conv weights become 12 component matrices
    G[k][ky] ([Cin, Cout], bf16) plus an identity block
  - device side per output chunk (8 rows x 128 cols = 2x512):
      E  = sum_ky G[0][ky]^T D[0] + I xy_even      (4 matmuls, PSUM)
      M1 = sum_ky G[1][ky]^T D[1]                  (3 matmuls)
      M2 = sum_ky G[2][ky]^T D[2]                  (3 matmuls)
      O  = sum_ky -G[3][ky]^T D[3] + I xy_odd      (4 matmuls)
    ScalarE: m1 = copy(M1) PSUM->SBUF (PSUM has one DVE read port, so
      every VectorE op below reads at most one PSUM operand)
    VectorE: t1 = E + m1; even = M2 + t1   (SBUF intermediates)
             u1 = O + m1; odd  = u1 - M2
    ScalarE: og = relu(x + bn_shift) -> bf16, DMA out per chunk
  - 14 N=512 matmuls per chunk vs 18 for direct conv: ~0.76x PE columns,
    no padded-width junk columns, and bf16 weights get the fast weight
    load path so LDWEIGHTS hides under the matmul stream
  - output is written as [even 8x64 | odd 8x64] blocks and re-interleaved
    host-side; all DMAs are contiguous >=2KB lines
"""

import numpy as np
import ml_dtypes

import concourse.bacc as bacc
import concourse.mybir as mybir
from concourse import tile
from concourse.bass_utils import run_bass_kernel_spmd

EPS = 1e-5

B, S, C1, C2, H, W = 4, 8, 64, 64, 128, 128
CC = C1 + C2               # 128 concat channels = out channels = partitions
N_CORES = 8
IMG_PER_CORE = (B * S) // N_CORES  # 4
HP = H + 2                 # padded rows (Winograd D spans all 130)
TW = W // 2                # 64 x-tiles per row
NB = 4                     # DMA bands per image
BR = H // NB               # 32 output rows per band
NQ = 16                    # chunks per image (8 rows each)
QR = H // NQ               # 8 rows per chunk

F32 = mybir.dt.float32
BF = mybir.dt.bfloat16
NPBF = ml_dtypes.bfloat16

_CACHE = {}


def _build_program():
    nc = bacc.Bacc(
        "TRN2", target_bir_lowering=False, debug=False, num_devices=N_CORES
    )
    # D components: [ch, comp, padded_row, xtile]; u half shared per core
    du_d = nc.dram_tensor("du", [C1, 4, HP, TW], BF, kind="ExternalInput")
    dv_d = nc.dram_tensor("dv", [IMG_PER_CORE, C2, 4, HP, TW], BF, kind="ExternalInput")
    # skip inputs split even/odd columns: [ch, eo, row, xtile]
    xu_d = nc.dram_tensor("xu", [C1, 2, H, TW], BF, kind="ExternalInput")
    xv_d = nc.dram_tensor("xv", [IMG_PER_CORE, C2, 2, H, TW], BF, kind="ExternalInput")
    w_d = nc.dram_tensor("w", [CC, 13 * CC], BF, kind="ExternalInput")
    sh_d = nc.dram_tensor("shift", [CC, 1], F32, kind="ExternalInput")
    # out: per chunk [even 8x64 | odd 8x64]; host re-interleaves
    o_d = nc.dram_tensor("o", [IMG_PER_CORE, CC, NQ * 2 * QR * TW], BF,
                         kind="ExternalOutput")

    with tile.TileContext(nc) as tc:
        with (
            tc.tile_pool(name="consts", bufs=1) as cpool,
            tc.tile_pool(name="xyb", bufs=4) as xpool,
            tc.tile_pool(name="scratch", bufs=3) as spool,
            tc.tile_pool(name="ostrip", bufs=6) as opool,
            tc.tile_pool(name="psum", bufs=8, space="PSUM") as ppool,
        ):
            w_r = cpool.tile([CC, 13 * CC], BF)
            nc.scalar.dma_start(w_r[:], w_d[:])
            sh_sb = cpool.tile([CC, 1], F32)
            nc.scalar.dma_start(sh_sb[:], sh_d[:])

            # two persistent whole-image D tiles (ping-pong); u half loaded
            # once per tile (all 4 images share the same u)
            d_a = cpool.tile([CC, 4, HP, TW], BF)
            d_b = cpool.tile([CC, 4, HP, TW], BF)

            # band row ranges; image 0 band 0 split finer so the first
            # chunk's operands land quickly
            def d_bands(first_img):
                r = []
                if first_img:
                    r.append((0, QR + 2))
                    r.append((QR + 2, BR + 2))
                else:
                    r.append((0, BR + 2))
                for bnd in range(1, NB):
                    r.append((bnd * BR + 2, (bnd + 1) * BR + 2))
                return r

            for img in range(IMG_PER_CORE):
                d_t = d_a if img % 2 == 0 else d_b
                first = img == 0
                # D loads: v half per image (sync queue), u half per tile
                # (scalar queue, images 0 and 1 only)
                for r0, r1 in d_bands(first):
                    nc.sync.dma_start(
                        d_t[C1:CC, :, r0:r1, :], dv_d[img, :, :, r0:r1, :]
                    )
                    if img < 2:
                        nc.scalar.dma_start(
                            d_t[0:C1, :, r0:r1, :], du_d[:, :, r0:r1, :]
                        )
                # skip-input bands (pooled)
                xybs = []
                for bnd in range(NB):
                    r0 = bnd * BR
                    xyb = xpool.tile([CC, 2, BR, TW], BF, tag="xy")
                    if first and bnd == 0:
                        for rr0, rr1 in ((0, QR), (QR, BR)):
                            nc.sync.dma_start(
                                xyb[C1:CC, :, rr0:rr1, :],
                                xv_d[img, :, :, r0 + rr0 : r0 + rr1, :],
                            )
                            nc.scalar.dma_start(
                                xyb[0:C1, :, rr0:rr1, :],
                                xu_d[:, :, r0 + rr0 : r0 + rr1, :],
                            )
                    else:
                        nc.sync.dma_start(
                            xyb[C1:CC, :, :, :], xv_d[img, :, :, r0 : r0 + BR, :]
                        )
                        nc.scalar.dma_start(
                            xyb[0:C1, :, :, :], xu_d[:, :, r0 : r0 + BR, :]
                        )
                    xybs.append(xyb)

                for bnd in range(NB):
                    xyb = xybs[bnd]
                    for q in range(NB):
                        rg = bnd * BR + q * QR      # global output row
                        rl = q * QR                  # row within band
                        ps = [ppool.tile([CC, 512], F32, tag="ps",
                                         name=f"ps{k}")
                              for k in range(4)]
                        # M1 first so the scalar copy starts early
                        for k in (1, 0, 2, 3):
                            for ky in range(3):
                                nc.tensor.matmul(
                                    ps[k][:],
                                    w_r[:, (3 * k + ky) * CC : (3 * k + ky + 1) * CC],
                                    d_t[:, k, rg + ky : rg + ky + QR, :],
                                    start=(ky == 0),
                                    stop=(ky == 2 and k not in (0, 3)),
                                )
                            if k == 0:
                                nc.tensor.matmul(
                                    ps[0][:],
                                    w_r[:, 12 * CC : 13 * CC],
                                    xyb[:, 0, rl : rl + QR, :],
                                    start=False, stop=True,
                                )
                            elif k == 3:
                                nc.tensor.matmul(
                                    ps[3][:],
                                    w_r[:, 12 * CC : 13 * CC],
                                    xyb[:, 1, rl : rl + QR, :],
                                    start=False, stop=True,
                                )
                        m1 = spool.tile([CC, 512], F32, tag="m1")
                        nc.scalar.copy(m1[:], ps[1][:])
                        t1 = spool.tile([CC, 512], F32, tag="t1")
                        u1 = spool.tile([CC, 512], F32, tag="u1")
                        nc.vector.tensor_add(t1[:], ps[0][:], m1[:])
                        nc.vector.tensor_add(u1[:], ps[3][:], m1[:])
                        # odd -> ps[3] (reads ps[2] before even overwrites it)
                        nc.vector.tensor_sub(ps[3][:], u1[:], ps[2][:])
                        # even -> ps[2]
                        nc.vector.tensor_add(ps[2][:], ps[2][:], t1[:])
                        og = opool.tile([CC, 2, QR, TW], BF, tag="og")
                        nc.scalar.activation(
                            og[:, 1].opt(), ps[3][:],
                            mybir.ActivationFunctionType.Relu,
                            bias=sh_sb[:], scale=1.0,
                        )
                        nc.scalar.activation(
                            og[:, 0].opt(), ps[2][:],
                            mybir.ActivationFunctionType.Relu,
                            bias=sh_sb[:], scale=1.0,
                        )
                        ci = bnd * NB + q
                        oeng = nc.sync if img == IMG_PER_CORE - 1 else nc.gpsimd
                        oeng.dma_start(
                            o_d[img, :, ci * 1024 : (ci + 1) * 1024],
                            og[:, :, :, :],
                        )
    nc.compile()
    return nc


def _get_program():
    if "nc" not in _CACHE:
        _CACHE["nc"] = _build_program()
    return _CACHE["nc"]


def _prep_inputs(u, v, conv_w, bn_gamma, bn_beta, bn_mean, bn_var):
    u = np.asarray(u, dtype=np.float32)
    v = np.asarray(v, dtype=np.float32)
    conv_w = np.asarray(conv_w, dtype=np.float32)
    bn_gamma = np.asarray(bn_gamma, dtype=np.float32)
    bn_beta = np.asarray(bn_beta, dtype=np.float32)
    bn_mean = np.asarray(bn_mean, dtype=np.float32)
    bn_var = np.asarray(bn_var, dtype=np.float32)

    scale = bn_gamma / np.sqrt(bn_var + EPS)
    shift = (bn_beta - bn_mean * scale).astype(np.float32).reshape(CC, 1)
    wsc = conv_w * scale[:, None, None, None]  # [out, in, ky, kx]
    W0, W1, W2 = wsc[..., 0], wsc[..., 1], wsc[..., 2]  # [out, in, ky]
    G = [W0, (W0 + W1 + W2) * 0.5, (W0 - W1 + W2) * 0.5, -W2]
    w_host = np.zeros((CC, 13 * CC), np.float32)
    for k in range(4):
        for ky in range(3):
            # lhsT block [in, out]
            w_host[:, (3 * k + ky) * CC : (3 * k + ky + 1) * CC] = G[k][:, :, ky].T
    w_host[:, 12 * CC : 13 * CC] = np.eye(CC, dtype=np.float32)
    w_host = w_host.astype(NPBF)

    def transform(x):
        """x: [C, H, W] fp32 -> (D [C,4,HP,TW] bf16, xy_eo [C,2,H,TW] bf16)."""
        C = x.shape[0]
        p = np.zeros((C, HP, W + 2), np.float32)
        p[:, 1 : 1 + H, 1 : 1 + W] = x
        p = p.astype(NPBF).astype(np.float32)
        D = np.empty((C, 4, HP, TW), np.float32)
        D[:, 0] = p[:, :, 0 : 2 * TW : 2] - p[:, :, 2 : 2 * TW + 2 : 2]
        D[:, 1] = p[:, :, 1 : 2 * TW + 1 : 2] + p[:, :, 2 : 2 * TW + 2 : 2]
        D[:, 2] = p[:, :, 2 : 2 * TW + 2 : 2] - p[:, :, 1 : 2 * TW + 1 : 2]
        D[:, 3] = p[:, :, 1 : 2 * TW + 1 : 2] - p[:, :, 3 : 2 * TW + 3 : 2]
        xy = np.empty((C, 2, H, TW), np.float32)
        xy[:, 0] = x[:, :, 0::2]
        xy[:, 1] = x[:, :, 1::2]
        return D.astype(NPBF), xy.astype(NPBF)

    in_maps = []
    du_cache = {}
    for m in range(N_CORES):
        b = m // 2
        s0 = (m % 2) * IMG_PER_CORE
        if b not in du_cache:
            du_cache[b] = transform(u[b, 0])
        du, xu = du_cache[b]
        dv = np.empty((IMG_PER_CORE, C2, 4, HP, TW), NPBF)
        xv = np.empty((IMG_PER_CORE, C2, 2, H, TW), NPBF)
        for i in range(IMG_PER_CORE):
            dv[i], xv[i] = transform(v[b, s0 + i])
        in_maps.append(
            {"du": du, "dv": dv, "xu": xu, "xv": xv, "w": w_host,
             "shift": shift}
        )
    return in_maps


def _run(inputs, trace=False):
    nc = _get_program()
    in_maps = _prep_inputs(**inputs)
    res = run_bass_kernel_spmd(nc, in_maps, list(range(N_CORES)), trace=trace)
    out = np.empty((B, 1, S, CC, H, W), np.float32)
    for m in range(N_CORES):
        b = m // 2
        s0 = (m % 2) * IMG_PER_CORE
        o = np.asarray(res.results[m]["o"]).astype(np.float32)
        # [img, CC, chunk, eo, row, xtile] -> [img, CC, chunk*row, xtile*2+eo]
        o = o.reshape(IMG_PER_CORE, CC, NQ, 2, QR, TW)
        o = o.transpose(0, 1, 2, 4, 5, 3).reshape(IMG_PER_CORE, CC, H, W)
        out[b, 0, s0 : s0 + IMG_PER_CORE] = o
    return out, res


def kernel(u, v, conv_w, bn_gamma, bn_beta, bn_mean, bn_var):
    out, _ = _run(
        dict(u=u, v=v, conv_w=conv_w, bn_gamma=bn_gamma, bn_beta=bn_beta,
             bn_mean=bn_mean, bn_var=bn_var)
    )
    return out


# revision 10
# speedup vs baseline: 1.2843x; 1.1059x over previous
"""CrossConv2d (concat -> 3x3 conv -> BN -> +skip -> ReLU) on 8 Trainium2 cores.

Data-parallel over the fused (b*s)=32 batch axis: 4 images per core.

1D Winograd F(2,3) along x, direct 3-tap accumulation along y, bf16:
  - host side: inputs are padded, cast to bf16, and x-transformed into
    4 Winograd components D[k] (one subtract/add per element); the
    BN-scale-folded conv weights become 12 component matrices
    G[k][ky] ([Cin, Cout], bf16) plus an identity block
  - device side per output chunk (8 rows x 128 cols = 2x512):
      E  = sum_ky G[0][ky]^T D[0] + I xy_even      (4 matmuls, PSUM)
      M1 = sum_ky G[1][ky]^T D[1]                  (3 matmuls)
      M2 = sum_ky G[2][ky]^T D[2]                  (3 matmuls)
      O  = sum_ky -G[3][ky]^T D[3] + I xy_odd      (4 matmuls)
    ScalarE: m1 = copy(M1) PSUM->SBUF (PSUM has one DVE read port, so
      every VectorE op below reads at most one PSUM operand)
    VectorE: t1 = E + m1; even = M2 + t1 -> PSUM[2]
             u1 = O + m1; odd  = u1 - M2 -> PSUM[3]
    ScalarE: og = relu(x + bn_shift) -> bf16, DMA out per 4 chunks
  - 14 N=512 matmuls per chunk vs 18 for direct conv: ~0.76x PE columns,
    no padded-width junk columns, and bf16 weights get the fast weight
    load path so LDWEIGHTS hides under the matmul stream
  - DMA is co-limiting (~300 GB/s aggregate): D's u half and the skip
    u half are loaded once (single D tile + persistent per-band xy
    tiles; per-image v halves overwrite with band-granular deps),
    outputs batch 4 chunks per DMA for 8KB contiguous lines
  - output is written as [even 8x64 | odd 8x64] blocks and re-interleaved
    host-side
"""

import numpy as np
import ml_dtypes

import concourse.bacc as bacc
import concourse.mybir as mybir
from concourse import tile
from concourse.bass_utils import run_bass_kernel_spmd

EPS = 1e-5

B, S, C1, C2, H, W = 4, 8, 64, 64, 128, 128
CC = C1 + C2               # 128 concat channels = out channels = partitions
N_CORES = 8
IMG_PER_CORE = (B * S) // N_CORES  # 4
HP = H + 2                 # padded rows (Winograd D spans all 130)
TW = W // 2                # 64 x-tiles per row
NB = 4                     # DMA bands per image
BR = H // NB               # 32 output rows per band
NQ = 16                    # chunks per image (8 rows each)
QR = H // NQ               # 8 rows per chunk

F32 = mybir.dt.float32
BF = mybir.dt.bfloat16
NPBF = ml_dtypes.bfloat16

_CACHE = {}


def _build_program():
    nc = bacc.Bacc(
        "TRN2", target_bir_lowering=False, debug=False, num_devices=N_CORES
    )
    # D components: [ch, comp, padded_row, xtile]; u half shared per core
    du_d = nc.dram_tensor("du", [C1, 4, HP, TW], BF, kind="ExternalInput")
    dv_d = nc.dram_tensor("dv", [IMG_PER_CORE, C2, 4, HP, TW], BF, kind="ExternalInput")
    # skip inputs split even/odd columns: [ch, eo, row, xtile]
    xu_d = nc.dram_tensor("xu", [C1, 2, H, TW], BF, kind="ExternalInput")
    xv_d = nc.dram_tensor("xv", [IMG_PER_CORE, C2, 2, H, TW], BF, kind="ExternalInput")
    w_d = nc.dram_tensor("w", [CC, 13 * CC], BF, kind="ExternalInput")
    sh_d = nc.dram_tensor("shift", [CC, 1], F32, kind="ExternalInput")
    # out: per chunk [even 8x64 | odd 8x64]; host re-interleaves
    o_d = nc.dram_tensor("o", [IMG_PER_CORE, CC, NQ * 2 * QR * TW], BF,
                         kind="ExternalOutput")

    with tile.TileContext(nc) as tc:
        with (
            tc.tile_pool(name="consts", bufs=1) as cpool,
            tc.tile_pool(name="scratch", bufs=3) as spool,
            tc.tile_pool(name="og4", bufs=3) as o4pool,
            tc.tile_pool(name="og1", bufs=4) as o1pool,
            tc.tile_pool(name="psum", bufs=8, space="PSUM") as ppool,
        ):
            w_r = cpool.tile([CC, 13 * CC], BF)
            nc.scalar.dma_start(w_r[:], w_d[:])
            sh_sb = cpool.tile([CC, 1], F32)
            nc.scalar.dma_start(sh_sb[:], sh_d[:])

            # single whole-image D tile: u half loaded once; v half
            # overwritten per image in bands (deps are band-granular, so
            # image i+1's loads overlap image i's tail compute)
            d_t = cpool.tile([CC, 4, HP, TW], BF)
            # persistent per-band skip tiles, same u/v scheme
            xyb = [cpool.tile([CC, 2, BR, TW], BF, name=f"xyb{b}")
                   for b in range(NB)]

            # band row ranges in padded-D space; image 0 band 0 split
            # finer so the first chunk's operands land quickly
            def d_bands(first_img):
                r = [(0, QR + 2), (QR + 2, BR + 2)] if first_img else [(0, BR + 2)]
                for bnd in range(1, NB):
                    r.append((bnd * BR + 2, (bnd + 1) * BR + 2))
                return r

            for img in range(IMG_PER_CORE):
                first = img == 0
                # interleave D-v and xy-v issues band by band (sync queue);
                # u halves on the scalar queue, first image only
                dbs = d_bands(first)
                di = 0
                for bnd in range(NB):
                    while di < len(dbs):
                        r0, r1 = dbs[di]
                        if r0 >= (bnd + 1) * BR + 2:
                            break
                        nc.sync.dma_start(
                            d_t[C1:CC, :, r0:r1, :], dv_d[img, :, :, r0:r1, :]
                        )
                        if first:
                            nc.scalar.dma_start(
                                d_t[0:C1, :, r0:r1, :], du_d[:, :, r0:r1, :]
                            )
                        di += 1
                    r0 = bnd * BR
                    if first and bnd == 0:
                        for rr0, rr1 in ((0, QR), (QR, BR)):
                            nc.sync.dma_start(
                                xyb[bnd][C1:CC, :, rr0:rr1, :],
                                xv_d[img, :, :, r0 + rr0 : r0 + rr1, :],
                            )
                            nc.scalar.dma_start(
                                xyb[bnd][0:C1, :, rr0:rr1, :],
                                xu_d[:, :, r0 + rr0 : r0 + rr1, :],
                            )
                    else:
                        nc.sync.dma_start(
                            xyb[bnd][C1:CC, :, :, :],
                            xv_d[img, :, :, r0 : r0 + BR, :],
                        )
                        if first:
                            nc.scalar.dma_start(
                                xyb[bnd][0:C1, :, :, :],
                                xu_d[:, :, r0 : r0 + BR, :],
                            )

                last_img = img == IMG_PER_CORE - 1
                og4 = None
                for bnd in range(NB):
                    for q in range(NB):
                        rg = bnd * BR + q * QR      # global output row
                        rl = q * QR                  # row within band
                        ci = bnd * NB + q
                        ps = [ppool.tile([CC, 512], F32, tag="ps",
                                         name=f"ps{k}")
                              for k in range(4)]
                        # M1 first so the scalar copy starts early
                        for k in (1, 0, 2, 3):
                            for ky in range(3):
                                nc.tensor.matmul(
                                    ps[k][:],
                                    w_r[:, (3 * k + ky) * CC : (3 * k + ky + 1) * CC],
                                    d_t[:, k, rg + ky : rg + ky + QR, :],
                                    start=(ky == 0),
                                    stop=(ky == 2 and k not in (0, 3)),
                                )
                            if k == 0:
                                nc.tensor.matmul(
                                    ps[0][:],
                                    w_r[:, 12 * CC : 13 * CC],
                                    xyb[bnd][:, 0, rl : rl + QR, :],
                                    start=False, stop=True,
                                )
                            elif k == 3:
                                nc.tensor.matmul(
                                    ps[3][:],
                                    w_r[:, 12 * CC : 13 * CC],
                                    xyb[bnd][:, 1, rl : rl + QR, :],
                                    start=False, stop=True,
                                )
                        m1 = spool.tile([CC, 512], F32, tag="m1")
                        nc.scalar.copy(m1[:], ps[1][:])
                        t1 = spool.tile([CC, 512], F32, tag="t1")
                        u1 = spool.tile([CC, 512], F32, tag="u1")
                        nc.vector.tensor_add(t1[:], ps[0][:], m1[:])
                        nc.vector.tensor_add(u1[:], ps[3][:], m1[:])
                        # odd -> ps[3] (reads ps[2] before even overwrites)
                        nc.vector.tensor_sub(ps[3][:], u1[:], ps[2][:])
                        # even -> ps[2]
                        nc.vector.tensor_add(ps[2][:], ps[2][:], t1[:])
                        if last_img:
                            og = o1pool.tile([CC, 2, QR, TW], BF, tag="og1")
                            oge, ogo = og[:, 0], og[:, 1]
                        else:
                            if ci % 4 == 0:
                                og4 = o4pool.tile([CC, 4, 2, QR, TW], BF,
                                                  tag="og4")
                            oge, ogo = og4[:, ci % 4, 0], og4[:, ci % 4, 1]
                        nc.scalar.activation(
                            ogo.opt(), ps[3][:],
                            mybir.ActivationFunctionType.Relu,
                            bias=sh_sb[:], scale=1.0,
                        )
                        nc.scalar.activation(
                            oge.opt(), ps[2][:],
                            mybir.ActivationFunctionType.Relu,
                            bias=sh_sb[:], scale=1.0,
                        )
                        if last_img:
                            nc.sync.dma_start(
                                o_d[img, :, ci * 1024 : (ci + 1) * 1024],
                                og[:, :, :, :],
                            )
                        elif ci % 4 == 3:
                            g = ci // 4
                            nc.gpsimd.dma_start(
                                o_d[img, :, g * 4096 : (g + 1) * 4096],
                                og4[:, :, :, :, :],
                            )
    nc.compile()
    return nc


def _get_program():
    if "nc" not in _CACHE:
        _CACHE["nc"] = _build_program()
    return _CACHE["nc"]


def _prep_inputs(u, v, conv_w, bn_gamma, bn_beta, bn_mean, bn_var):
    u = np.asarray(u, dtype=np.float32)
    v = np.asarray(v, dtype=np.float32)
    conv_w = np.asarray(conv_w, dtype=np.float32)
    bn_gamma = np.asarray(bn_gamma, dtype=np.float32)
    bn_beta = np.asarray(bn_beta, dtype=np.float32)
    bn_mean = np.asarray(bn_mean, dtype=np.float32)
    bn_var = np.asarray(bn_var, dtype=np.float32)

    scale = bn_gamma / np.sqrt(bn_var + EPS)
    shift = (bn_beta - bn_mean * scale).astype(np.float32).reshape(CC, 1)
    wsc = conv_w * scale[:, None, None, None]  # [out, in, ky, kx]
    W0, W1, W2 = wsc[..., 0], wsc[..., 1], wsc[..., 2]  # [out, in, ky]
    G = [W0, (W0 + W1 + W2) * 0.5, (W0 - W1 + W2) * 0.5, -W2]
    w_host = np.zeros((CC, 13 * CC), np.float32)
    for k in range(4):
        for ky in range(3):
            # lhsT block [in, out]
            w_host[:, (3 * k + ky) * CC : (3 * k + ky + 1) * CC] = G[k][:, :, ky].T
    w_host[:, 12 * CC : 13 * CC] = np.eye(CC, dtype=np.float32)
    w_host = w_host.astype(NPBF)

    def transform(x):
        """x: [C, H, W] fp32 -> (D [C,4,HP,TW] bf16, xy_eo [C,2,H,TW] bf16)."""
        C = x.shape[0]
        p = np.zeros((C, HP, W + 2), np.float32)
        p[:, 1 : 1 + H, 1 : 1 + W] = x
        p = p.astype(NPBF).astype(np.float32)
        D = np.empty((C, 4, HP, TW), np.float32)
        D[:, 0] = p[:, :, 0 : 2 * TW : 2] - p[:, :, 2 : 2 * TW + 2 : 2]
        D[:, 1] = p[:, :, 1 : 2 * TW + 1 : 2] + p[:, :, 2 : 2 * TW + 2 : 2]
        D[:, 2] = p[:, :, 2 : 2 * TW + 2 : 2] - p[:, :, 1 : 2 * TW + 1 : 2]
        D[:, 3] = p[:, :, 1 : 2 * TW + 1 : 2] - p[:, :, 3 : 2 * TW + 3 : 2]
        xy = np.empty((C, 2, H, TW), np.float32)
        xy[:, 0] = x[:, :, 0::2]
        xy[:, 1] = x[:, :, 1::2]
        return D.astype(NPBF), xy.astype(NPBF)

    in_maps = []
    du_cache = {}
    for m in range(N_CORES):
        b = m // 2
        s0 = (m % 2) * IMG_PER_CORE
        if b not in du_cache:
            du_cache[b] = transform(u[b, 0])
        du, xu = du_cache[b]
        dv = np.empty((IMG_PER_CORE, C2, 4, HP, TW), NPBF)
        xv = np.empty((IMG_PER_CORE, C2, 2, H, TW), NPBF)
        for i in range(IMG_PER_CORE):
            dv[i], xv[i] = transform(v[b, s0 + i])
        in_maps.append(
            {"du": du, "dv": dv, "xu": xu, "xv": xv, "w": w_host,
             "shift": shift}
        )
    return in_maps


def _run(inputs, trace=False):
    nc = _get_program()
    in_maps = _prep_inputs(**inputs)
    res = run_bass_kernel_spmd(nc, in_maps, list(range(N_CORES)), trace=trace)
    out = np.empty((B, 1, S, CC, H, W), np.float32)
    for m in range(N_CORES):
        b = m // 2
        s0 = (m % 2) * IMG_PER_CORE
        o = np.asarray(res.results[m]["o"]).astype(np.float32)
        # [img, CC, chunk, eo, row, xtile] -> [img, CC, chunk*row, xtile*2+eo]
        o = o.reshape(IMG_PER_CORE, CC, NQ, 2, QR, TW)
        o = o.transpose(0, 1, 2, 4, 5, 3).reshape(IMG_PER_CORE, CC, H, W)
        out[b, 0, s0 : s0 + IMG_PER_CORE] = o
    return out, res


def kernel(u, v, conv_w, bn_gamma, bn_beta, bn_mean, bn_var):
    out, _ = _run(
        dict(u=u, v=v, conv_w=conv_w, bn_gamma=bn_gamma, bn_beta=bn_beta,
             bn_mean=bn_mean, bn_var=bn_var)
    )
    return out


# revision 14
# speedup vs baseline: 1.3382x; 1.0420x over previous
"""CrossConv2d (concat -> 3x3 conv -> BN -> +skip -> ReLU) on 8 Trainium2 cores.

Data-parallel over the fused (b*s)=32 batch axis: 4 images per core.

1D Winograd F(2,3) along x, direct 3-tap accumulation along y, bf16:
  - host side: inputs are padded, cast to bf16, and x-transformed into
    4 Winograd components D[k] (one subtract/add per element); the
    BN-scale-folded conv weights become 12 component matrices
    G[k][ky] ([Cin, Cout], bf16) plus an identity block
  - device side per output chunk (8 rows x 128 cols = 2x512):
      E  = sum_ky G[0][ky]^T D[0] + I xy_even      (4 matmuls, PSUM)
      M1 = sum_ky G[1][ky]^T D[1]                  (3 matmuls)
      M2 = sum_ky G[2][ky]^T D[2]                  (3 matmuls)
      O  = sum_ky -G[3][ky]^T D[3] + I xy_odd      (4 matmuls)
    ScalarE: m1 = copy(M1) PSUM->SBUF (PSUM has one DVE read port, so
      every VectorE op below reads at most one PSUM operand)
    VectorE: t1 = E + m1; even = M2 + t1 -> PSUM[2]
             u1 = O + m1; odd  = u1 - M2 -> PSUM[3]
    ScalarE: og = relu(x + bn_shift) -> bf16, DMA out per 4 chunks
  - 14 N=512 matmuls per chunk vs 18 for direct conv: ~0.76x PE columns,
    no padded-width junk columns, and bf16 weights get the fast weight
    load path so LDWEIGHTS hides under the matmul stream
  - DMA is co-limiting (~300 GB/s aggregate): D's u half and the skip
    u half are loaded once (single D tile + persistent per-band xy
    tiles; per-image v halves overwrite with band-granular deps),
    outputs batch 4 chunks per DMA for 8KB contiguous lines
  - output is written as [even 8x64 | odd 8x64] blocks and re-interleaved
    host-side
"""

import numpy as np
import ml_dtypes

import concourse.bacc as bacc
import concourse.mybir as mybir
from concourse import tile
from concourse.bass_utils import run_bass_kernel_spmd

EPS = 1e-5

B, S, C1, C2, H, W = 4, 8, 64, 64, 128, 128
CC = C1 + C2               # 128 concat channels = out channels = partitions
N_CORES = 8
IMG_PER_CORE = (B * S) // N_CORES  # 4
HP = H + 2                 # padded rows (Winograd D spans all 130)
TW = W // 2                # 64 x-tiles per row
NB = 4                     # DMA bands per image
BR = H // NB               # 32 output rows per band
NQ = 16                    # chunks per image (8 rows each)
QR = H // NQ               # 8 rows per chunk

F32 = mybir.dt.float32
BF = mybir.dt.bfloat16
NPBF = ml_dtypes.bfloat16

_CACHE = {}


def _build_program():
    nc = bacc.Bacc(
        "TRN2", target_bir_lowering=False, debug=False, num_devices=N_CORES
    )
    # D components: [ch, comp, padded_row, xtile]; u half shared per core
    du_d = nc.dram_tensor("du", [C1, 4, HP, TW], BF, kind="ExternalInput")
    dv_d = nc.dram_tensor("dv", [IMG_PER_CORE, C2, 4, HP, TW], BF, kind="ExternalInput")
    # skip inputs split even/odd columns: [ch, eo, row, xtile]
    xu_d = nc.dram_tensor("xu", [C1, 2, H, TW], BF, kind="ExternalInput")
    xv_d = nc.dram_tensor("xv", [IMG_PER_CORE, C2, 2, H, TW], BF, kind="ExternalInput")
    w_d = nc.dram_tensor("w", [CC, 13 * CC], BF, kind="ExternalInput")
    sh_d = nc.dram_tensor("shift", [CC, 1], F32, kind="ExternalInput")
    # out: per chunk [even 8x64 | odd 8x64]; host re-interleaves
    o_d = nc.dram_tensor("o", [IMG_PER_CORE, CC, NQ * 2 * QR * TW], BF,
                         kind="ExternalOutput")

    with tile.TileContext(nc) as tc:
        with (
            tc.tile_pool(name="consts", bufs=1) as cpool,
            tc.tile_pool(name="scratch", bufs=3) as spool,
            tc.tile_pool(name="og4", bufs=3) as o4pool,
            tc.tile_pool(name="og1", bufs=4) as o1pool,
            tc.tile_pool(name="psum", bufs=8, space="PSUM") as ppool,
        ):
            w_r = cpool.tile([CC, 13 * CC], BF)
            nc.scalar.dma_start(w_r[:], w_d[:])
            sh_sb = cpool.tile([CC, 1], F32)
            nc.scalar.dma_start(sh_sb[:], sh_d[:])

            # single whole-image D tile: u half loaded once; v half
            # overwritten per image in bands (deps are band-granular, so
            # image i+1's loads overlap image i's tail compute)
            d_t = cpool.tile([CC, 4, HP, TW], BF)
            # persistent per-band skip tiles, same u/v scheme
            xyb = [cpool.tile([CC, 2, BR, TW], BF, name=f"xyb{b}")
                   for b in range(NB)]

            # band row ranges in padded-D space; image 0 band 0 split
            # finer so the first chunk's operands land quickly
            def d_bands(first_img):
                r = [(0, QR + 2), (QR + 2, BR + 2)] if first_img else [(0, BR + 2)]
                for bnd in range(1, NB):
                    r.append((bnd * BR + 2, (bnd + 1) * BR + 2))
                return r

            def u_band_issues(bnd):
                """u-half loads for band bnd (image 0 only) on scalar."""
                dr0 = bnd * BR + 2 if bnd else 0
                dr1 = (bnd + 1) * BR + 2
                nc.scalar.dma_start(
                    d_t[0:C1, :, dr0:dr1, :], du_d[:, :, dr0:dr1, :]
                )
                r0 = bnd * BR
                nc.scalar.dma_start(
                    xyb[bnd][0:C1, :, :, :], xu_d[:, :, r0 : r0 + BR, :]
                )

            for img in range(IMG_PER_CORE):
                first = img == 0
                # v halves band by band on the sync queue, interleaved so
                # the first chunk's operands land first; u halves (image 0
                # only) on scalar — band 0 up front, bands 1-3 woven into
                # the first chunks so they don't head-of-line-block the
                # chunk pipeline's scalar ops
                dbs = d_bands(first)
                di = 0
                for bnd in range(NB):
                    xps = ((0, QR), (QR, BR)) if first and bnd == 0 else ((0, BR),)
                    xi = 0
                    r0 = bnd * BR
                    emitted = []
                    while di < len(dbs):
                        dr0, dr1 = dbs[di]
                        if dr0 >= (bnd + 1) * BR + 2:
                            break
                        emitted.append(("d", dr0, dr1))
                        di += 1
                        if xi < len(xps):
                            emitted.append(("x", *xps[xi]))
                            xi += 1
                    emitted.extend(("x", *xps[i]) for i in range(xi, len(xps)))
                    for kind, a0, a1 in emitted:
                        if kind == "d":
                            nc.sync.dma_start(
                                d_t[C1:CC, :, a0:a1, :],
                                dv_d[img, :, :, a0:a1, :],
                            )
                            if first and bnd == 0:
                                nc.scalar.dma_start(
                                    d_t[0:C1, :, a0:a1, :],
                                    du_d[:, :, a0:a1, :],
                                )
                        else:
                            nc.sync.dma_start(
                                xyb[bnd][C1:CC, :, a0:a1, :],
                                xv_d[img, :, :, r0 + a0 : r0 + a1, :],
                            )
                            if first and bnd == 0:
                                nc.scalar.dma_start(
                                    xyb[bnd][0:C1, :, a0:a1, :],
                                    xu_d[:, :, r0 + a0 : r0 + a1, :],
                                )

                last_img = img == IMG_PER_CORE - 1
                og4 = None
                for bnd in range(NB):
                    for q in range(NB):
                        rg = bnd * BR + q * QR      # global output row
                        rl = q * QR                  # row within band
                        ci = bnd * NB + q
                        ps = [ppool.tile([CC, 512], F32, tag="ps",
                                         name=f"ps{k}")
                              for k in range(4)]
                        # M1 first so the scalar copy starts early
                        for k in (1, 0, 2, 3):
                            for ky in range(3):
                                nc.tensor.matmul(
                                    ps[k][:],
                                    w_r[:, (3 * k + ky) * CC : (3 * k + ky + 1) * CC],
                                    d_t[:, k, rg + ky : rg + ky + QR, :],
                                    start=(ky == 0),
                                    stop=(ky == 2 and k not in (0, 3)),
                                )
                            if k == 0:
                                nc.tensor.matmul(
                                    ps[0][:],
                                    w_r[:, 12 * CC : 13 * CC],
                                    xyb[bnd][:, 0, rl : rl + QR, :],
                                    start=False, stop=True,
                                )
                            elif k == 3:
                                nc.tensor.matmul(
                                    ps[3][:],
                                    w_r[:, 12 * CC : 13 * CC],
                                    xyb[bnd][:, 1, rl : rl + QR, :],
                                    start=False, stop=True,
                                )
                        m1 = spool.tile([CC, 512], F32, tag="m1")
                        nc.scalar.copy(m1[:], ps[1][:])
                        t1 = spool.tile([CC, 512], F32, tag="t1")
                        u1 = spool.tile([CC, 512], F32, tag="u1")
                        nc.vector.tensor_add(t1[:], ps[0][:], m1[:])
                        nc.vector.tensor_add(u1[:], ps[3][:], m1[:])
                        # odd -> ps[3] (reads ps[2] before even overwrites)
                        nc.vector.tensor_sub(ps[3][:], u1[:], ps[2][:])
                        # even -> ps[2]
                        nc.vector.tensor_add(ps[2][:], ps[2][:], t1[:])
                        if last_img:
                            og = o1pool.tile([CC, 2, QR, TW], BF, tag="og1")
                            oge, ogo = og[:, 0], og[:, 1]
                        else:
                            if ci % 4 == 0:
                                og4 = o4pool.tile([CC, 4, 2, QR, TW], BF,
                                                  tag="og4")
                            oge, ogo = og4[:, ci % 4, 0], og4[:, ci % 4, 1]
                        nc.scalar.activation(
                            ogo.opt(), ps[3][:],
                            mybir.ActivationFunctionType.Relu,
                            bias=sh_sb[:], scale=1.0,
                        )
                        nc.scalar.activation(
                            oge.opt(), ps[2][:],
                            mybir.ActivationFunctionType.Relu,
                            bias=sh_sb[:], scale=1.0,
                        )
                        # weave image-0 u-half loads for bands 1-3 between
                        # the first chunks' scalar ops
                        if first and ci < NB - 1:
                            u_band_issues(ci + 1)
                        if last_img:
                            oeng = nc.sync if ci % 2 == 1 else nc.scalar
                            oeng.dma_start(
                                o_d[img, :, ci * 1024 : (ci + 1) * 1024],
                                og[:, :, :, :],
                            )
                        elif ci % 4 == 3:
                            g = ci // 4
                            nc.gpsimd.dma_start(
                                o_d[img, :, g * 4096 : (g + 1) * 4096],
                                og4[:, :, :, :, :],
                            )
    nc.compile()
    return nc


def _get_program():
    if "nc" not in _CACHE:
        _CACHE["nc"] = _build_program()
    return _CACHE["nc"]


def _prep_inputs(u, v, conv_w, bn_gamma, bn_beta, bn_mean, bn_var):
    u = np.asarray(u, dtype=np.float32)
    v = np.asarray(v, dtype=np.float32)
    conv_w = np.asarray(conv_w, dtype=np.float32)
    bn_gamma = np.asarray(bn_gamma, dtype=np.float32)
    bn_beta = np.asarray(bn_beta, dtype=np.float32)
    bn_mean = np.asarray(bn_mean, dtype=np.float32)
    bn_var = np.asarray(bn_var, dtype=np.float32)

    scale = bn_gamma / np.sqrt(bn_var + EPS)
    shift = (bn_beta - bn_mean * scale).astype(np.float32).reshape(CC, 1)
    wsc = conv_w * scale[:, None, None, None]  # [out, in, ky, kx]
    W0, W1, W2 = wsc[..., 0], wsc[..., 1], wsc[..., 2]  # [out, in, ky]
    G = [W0, (W0 + W1 + W2) * 0.5, (W0 - W1 + W2) * 0.5, -W2]
    w_host = np.zeros((CC, 13 * CC), np.float32)
    for k in range(4):
        for ky in range(3):
            # lhsT block [in, out]
            w_host[:, (3 * k + ky) * CC : (3 * k + ky + 1) * CC] = G[k][:, :, ky].T
    w_host[:, 12 * CC : 13 * CC] = np.eye(CC, dtype=np.float32)
    w_host = w_host.astype(NPBF)

    def transform(x):
        """x: [C, H, W] fp32 -> (D [C,4,HP,TW] bf16, xy_eo [C,2,H,TW] bf16)."""
        C = x.shape[0]
        p = np.zeros((C, HP, W + 2), np.float32)
        p[:, 1 : 1 + H, 1 : 1 + W] = x
        p = p.astype(NPBF).astype(np.float32)
        D = np.empty((C, 4, HP, TW), np.float32)
        D[:, 0] = p[:, :, 0 : 2 * TW : 2] - p[:, :, 2 : 2 * TW + 2 : 2]
        D[:, 1] = p[:, :, 1 : 2 * TW + 1 : 2] + p[:, :, 2 : 2 * TW + 2 : 2]
        D[:, 2] = p[:, :, 2 : 2 * TW + 2 : 2] - p[:, :, 1 : 2 * TW + 1 : 2]
        D[:, 3] = p[:, :, 1 : 2 * TW + 1 : 2] - p[:, :, 3 : 2 * TW + 3 : 2]
        xy = np.empty((C, 2, H, TW), np.float32)
        xy[:, 0] = x[:, :, 0::2]
        xy[:, 1] = x[:, :, 1::2]
        return D.astype(NPBF), xy.astype(NPBF)

    in_maps = []
    du_cache = {}
    for m in range(N_CORES):
        b = m // 2
        s0 = (m % 2) * IMG_PER_CORE
        if b not in du_cache:
            du_cache[b] = transform(u[b, 0])
        du, xu = du_cache[b]
        dv = np.empty((IMG_PER_CORE, C2, 4, HP, TW), NPBF)
        xv = np.empty((IMG_PER_CORE, C2, 2, H, TW), NPBF)
        for i in range(IMG_PER_CORE):
            dv[i], xv[i] = transform(v[b, s0 + i])
        in_maps.append(
            {"du": du, "dv": dv, "xu": xu, "xv": xv, "w": w_host,
             "shift": shift}
        )
    return in_maps


def _run(inputs, trace=False):
    nc = _get_program()
    in_maps = _prep_inputs(**inputs)
    res = run_bass_kernel_spmd(nc, in_maps, list(range(N_CORES)), trace=trace)
    out = np.empty((B, 1, S, CC, H, W), np.float32)
    for m in range(N_CORES):
        b = m // 2
        s0 = (m % 2) * IMG_PER_CORE
        o = np.asarray(res.results[m]["o"]).astype(np.float32)
        # [img, CC, chunk, eo, row, xtile] -> [img, CC, chunk*row, xtile*2+eo]
        o = o.reshape(IMG_PER_CORE, CC, NQ, 2, QR, TW)
        o = o.transpose(0, 1, 2, 4, 5, 3).reshape(IMG_PER_CORE, CC, H, W)
        out[b, 0, s0 : s0 + IMG_PER_CORE] = o
    return out, res


def kernel(u, v, conv_w, bn_gamma, bn_beta, bn_mean, bn_var):
    out, _ = _run(
        dict(u=u, v=v, conv_w=conv_w, bn_gamma=bn_gamma, bn_beta=bn_beta,
             bn_mean=bn_mean, bn_var=bn_var)
    )
    return out
